# revision 58
# baseline (speedup 1.0000x reference)
"""Multi-head attention (B=16, T=1024, D=768, H=12) on 8 TRN2 NeuronCores.

Strategy: pure data parallelism over the batch dim (2 batches per core, no
collectives). Per core, a Tile kernel computes the full attention block:

  qkv = x @ Wqkv.T + b            (q,k produced transposed [o, T]; v normal [T, o])
  scoresT = (k_h qT_h) * scale    ([j, i] layout; the head-pair's two K=64
                                   matmuls land in row groups 0-1 / 2-3 and
                                   run concurrently in the PE array)
  expT = exp(scoresT)             (ScalarE, one [128, 1024] call per head-pair
                                   covering both heads' PSUM banks)
  outT_aug = v_aug.T? PV matmul   (v with appended ones col -> rows 0..63 = out,
                                   row 64 = softmax denominators)
  outT = outT_aug[:64] / sums     (SWDGE den row to partition 0, single-pass
                                   DVE approx-recip, Q7 partition_broadcast,
                                   DVE muls — staged over 4 slots so no queue
                                   head-of-line blocks on another engine.
                                   NOTE: gpsimd + custom-DVE ops misread
                                   partition-OFFSET sources on HW, so every
                                   recip/broadcast source must be base-0.)
  y = outT.T @ WprojT + b         (normal [t, e] layout, contiguous DMA out)

Startup: the aggregate HBM bandwidth of 8 cores loading duplicated weights
is the gate. Every big input is host-packed [128, n_dc*C] (contraction
chunks contiguous per partition) so each tensor loads as ONE DMA with
3-12 KB per-partition packets — per-packet overhead otherwise halves the
effective rate. The qkv weights are additionally permuted into hot blocks
(pair-0 q/k, then v) DMA'd first (dc-halved for finer first-matmul deps),
then x0 halves, then everything else in first-use order on the single sync
ring (a second DMA ring only steals engine bandwidth from the priority
order). The attention pipeline starts ~19us in instead of ~39us.

Scheduling: the attention inner loop is ScalarE-gated (exp), so all dense
GEMM work (qkv production for both batches, both output projections) is
interleaved into the attention emission as deadline-paced PE filler — the
PE stays busy and HAM-warm instead of micro-idling into the throttled
1.2 GHz clock. The PV matmuls trail their scores/exp by a 3-deep stash
(the drain loop must run n_it+depth slots — a shallower bound silently
drops the last PV), with double-buffered 2-bank score PSUM tiles so no
matmul waits on the exp that drains the other buffer. Both batches run
i0-outer so each batch's first-half output projection becomes late filler
inside its own attention phase; fillers drain at 1 group/slot (2 when
overdue) to spread coverage, and six batch-0 projection groups are held
back to cover the final normalization-chain latency in the drain.

All matmuls run in bf16 with f32 PSUM accumulation; f32 -> bf16 casts happen
on-chip (DVE). Softmax max-subtraction is skipped: scores are ~N(0,1) here so
exp() cannot overflow f32/bf16. Output is stored bf16 and widened on host.
"""

import numpy as np

import concourse.bass as bass
import concourse.mybir as mybir
import concourse.tile as tile
from concourse import bacc
from concourse.bass_utils import run_bass_kernel_spmd

F32 = mybir.dt.float32
BF16 = mybir.dt.bfloat16

N_CORES = 8
B = 16
T = 1024
NH = 12
HD = 64
DIM = NH * HD
B_LOC = B // N_CORES
TC = 512  # free-dim chunk (one PSUM bank of f32)


def build_nc(b_loc=B_LOC, t=T, nh=NH):
    assert nh % 2 == 0
    dim = nh * HD
    n_dc = dim // 128      # contraction chunks over dim
    n_qk = 2 * nh // 2     # o-tiles covering q and k rows (pair-granular)
    n_tt = t // 128        # t tiles
    n_hp = nh // 2
    scale = HD ** -0.5

    nc = bacc.Bacc()

    # All big inputs are host-packed [128, n_dc*C]: the 6 contraction
    # chunks lie contiguous per partition, so each tensor is ONE DMA with
    # 3-12 KB per-partition packets instead of 6 DMAs of 0.5-1.5 KB
    # packets — the startup is aggregate-HBM-bound across 8 cores and
    # packet overhead was halving the effective rate.
    # x: [b, half-t, 128, n_dc*512]; wqkv columns host-permuted:
    # hqk = pair-0 q/k, hv = v, ra = pairs 1-2, rb = pairs 3-5
    # (each pair = [q 128 | k 128])
    xP_d = nc.declare_dram_parameter("xP", [b_loc, 2, 128, n_dc * TC], BF16,
                                     isOutput=False)
    whqk_d = nc.declare_dram_parameter("w_hqk", [128, n_dc * 256], BF16, isOutput=False)
    whv_d = nc.declare_dram_parameter("w_hv", [128, n_dc * dim], BF16, isOutput=False)
    wra_d = nc.declare_dram_parameter("w_ra", [128, n_dc * 512], BF16, isOutput=False)
    wrb_d = nc.declare_dram_parameter("w_rb", [128, n_dc * 768], BF16, isOutput=False)
    wp_d = nc.declare_dram_parameter("w_projT", [128, n_dc * dim], BF16, isOutput=False)
    bqk_d = nc.declare_dram_parameter("b_qkT", [128, 2 * n_hp], F32, isOutput=False)
    bv_d = nc.declare_dram_parameter("b_v", [128, dim], BF16, isOutput=False)
    bp_d = nc.declare_dram_parameter("b_proj", [128, dim], BF16, isOutput=False)
    out_d = nc.declare_dram_parameter("out", [b_loc, t, dim], BF16, isOutput=True)

    import contextlib
    with tile.TileContext(nc) as tc, contextlib.ExitStack() as stack:
        ep = stack.enter_context
        p_whqk = ep(tc.tile_pool(name="whqk", bufs=1))
        p_whv = ep(tc.tile_pool(name="whv", bufs=1))
        p_wra = ep(tc.tile_pool(name="wra", bufs=1))
        p_wrb = ep(tc.tile_pool(name="wrb", bufs=1))
        p_wp = ep(tc.tile_pool(name="wp", bufs=1))
        p_x = ep(tc.tile_pool(name="xbf", bufs=2 * 2))
        p_qk = ep(tc.tile_pool(name="qk", bufs=2 * 2 * n_hp))
        p_v = ep(tc.tile_pool(name="v", bufs=2 * n_tt))
        p_out = ep(tc.tile_pool(name="outT", bufs=2 * n_hp))
        p_exp = ep(tc.tile_pool(name="expT", bufs=5))
        p_b = ep(tc.tile_pool(name="bias", bufs=1))
        p_y = ep(tc.tile_pool(name="y", bufs=3))
        p_sm = ep(tc.tile_pool(name="small", bufs=2))
        p_den = ep(tc.tile_pool(name="den", bufs=2))
        p_bc = ep(tc.tile_pool(name="bc", bufs=3))
        p_cp = ep(tc.tile_pool(name="pocp", bufs=6))
        ps_sc = ep(tc.tile_pool(name="pssc", bufs=2, space="PSUM"))
        ps_o = ep(tc.tile_pool(name="pso", bufs=2, space="PSUM"))
        ps_mm = ep(tc.tile_pool(name="psmm", bufs=2, space="PSUM"))
        if True:
            # ---- DMA issue order = arrival priority ----
            # 1. tiny q/k bias (warms exp table dependency too)
            b_qk_sb = p_b.tile([128, 2 * n_hp], F32, tag="bqk")
            nc.sync.dma_start(b_qk_sb[:], bqk_d[:, :])

            # 2. pair-0 q/k weights then x0 first half, split in dc-halves
            #    so the first accumulation chain starts on the first half
            x_t = {b: [None, None] for b in range(b_loc)}
            hw = n_dc // 2
            whqk_sb = p_whqk.tile([128, n_dc * 256], BF16, tag="whqk")
            xb = p_x.tile([128, n_dc * TC], BF16, tag="xbf", name="xb")
            nc.sync.dma_start(whqk_sb[:, 0:hw * 256], whqk_d[:, 0:hw * 256])
            nc.sync.dma_start(xb[:, 0:hw * TC], xP_d[0, 0, :, 0:hw * TC])
            nc.sync.dma_start(whqk_sb[:, hw * 256:], whqk_d[:, hw * 256:])
            nc.sync.dma_start(xb[:, hw * TC:], xP_d[0, 0, :, hw * TC:])
            x_t[0][0] = xb
            # 3. v weights + v bias (PV consumes v from attention slot 2 on)
            whv_sb = p_whv.tile([128, n_dc * dim], BF16, tag="whv")
            nc.sync.dma_start(whv_sb[:], whv_d[:, :])
            b_v_sb = p_b.tile([128, dim], BF16, tag="bv")
            nc.sync.dma_start(b_v_sb[:], bv_d[:, :])
            # 4. x batch 0 second half-t (scores jt>=4 / k(TC) group)
            xb = p_x.tile([128, n_dc * TC], BF16, tag="xbf", name="xb")
            nc.sync.dma_start(xb[:], xP_d[0, 1, :, :])
            x_t[0][1] = xb
            # 5. rest-a weights (pairs 1-2, consumed by B0 fillers slot 10+)
            wra_sb = p_wra.tile([128, n_dc * 512], BF16, tag="wra")
            nc.sync.dma_start(wra_sb[:], wra_d[:, :])
            # 6. x batch 1 (batch-1 qkv fillers from B0 slot ~16 on)
            for half in range(2):
                xb = p_x.tile([128, n_dc * TC], BF16, tag="xbf", name="xb")
                nc.sync.dma_start(xb[:], xP_d[1, half, :, :])
                x_t[1][half] = xb
            # 7. rest-b weights (pairs 3-5)
            wrb_sb = p_wrb.tile([128, n_dc * 768], BF16, tag="wrb")
            nc.sync.dma_start(wrb_sb[:], wrb_d[:, :])
            # 8. proj weights
            wp_sb = p_wp.tile([128, n_dc * dim], BF16, tag="wp")
            nc.sync.dma_start(wp_sb[:], wp_d[:, :])
            # 9. proj bias (first needed by proj fillers in phase B1)
            b_p_sb = p_b.tile([128, dim], BF16, tag="bp")
            nc.sync.dma_start(b_p_sb[:], bp_d[:, :])

            # warm the ScalarE exp table set while the DMAs run (first
            # ACTIVATE pays ~2.7us for the table DMA otherwise mid-attention).
            # Emitted AFTER the scalar-ring DMA doorbells: it waits on the
            # b_qk load, and anything queued behind it on the scalar ring
            # would inherit that wait.
            warm = p_sm.tile([1, 2 * n_hp], BF16, tag="warm")
            nc.scalar.activation(
                warm[:], b_qk_sb[0:1, :], mybir.ActivationFunctionType.Exp,
                scale=0.0,
            )

            def qk_w(dc, hp, which):
                # weight chunk [128, 128] for q/k of pair hp
                off = 0 if which == "q" else 128
                if hp == 0:
                    o = dc * 256 + off
                    return whqk_sb[:, o:o + 128]
                if hp <= 2:
                    o = dc * 512 + (hp - 1) * 256 + off
                    return wra_sb[:, o:o + 128]
                o = dc * 768 + (hp - 3) * 256 + off
                return wrb_sb[:, o:o + 128]

            qk_t = {b: {} for b in range(b_loc)}
            v_t = {b: [None] * n_tt for b in range(b_loc)}
            outT = {b: [None] * n_hp for b in range(b_loc)}

            def emit_qk_group(b, hp, which, i0):
                # q/k pair-tile chunk, transposed layout [o, t]
                key = (hp, which)
                if key not in qk_t[b]:
                    qk_t[b][key] = p_qk.tile([128, t], BF16, tag="qk", name="qt")
                ps = ps_mm.tile([128, TC], F32, tag="mm", name="ps")
                for dc in range(n_dc):
                    nc.tensor.matmul(
                        ps[:],
                        lhsT=qk_w(dc, hp, which),
                        rhs=x_t[b][i0 // TC][:, dc * TC:(dc + 1) * TC],
                        start=(dc == 0),
                        stop=(dc == n_dc - 1),
                    )
                bcol = hp if which == "q" else n_hp + hp
                nc.vector.tensor_scalar_add(
                    qk_t[b][key][:, i0:i0 + TC], ps[:], b_qk_sb[:, bcol:bcol + 1]
                )

            def emit_v_group(b, tt, o0):
                # v t-tile chunk, normal layout [t, o'] with ones col per head
                if v_t[b][tt] is None:
                    vt = p_v.tile([128, nh * 65], BF16, tag="v", name="vt")
                    v3 = vt[:].rearrange("p (h c) -> p h c", c=65)
                    nc.vector.memset(v3[:, :, 64:65], 1.0)
                    v_t[b][tt] = vt
                vt = v_t[b][tt]
                v3 = vt[:].rearrange("p (h c) -> p h c", c=65)
                oc = min(TC, dim - o0)
                h0 = o0 // 64
                nhc = oc // 64
                ps = ps_mm.tile([128, oc], F32, tag="mm", name="ps")
                for dc in range(n_dc):
                    nc.tensor.matmul(
                        ps[:],
                        lhsT=x_t[b][tt // 4][:, dc * TC + (tt % 4) * 128:
                                             dc * TC + (tt % 4 + 1) * 128],
                        rhs=whv_sb[:, dc * dim + o0:dc * dim + o0 + oc],
                        start=(dc == 0),
                        stop=(dc == n_dc - 1),
                    )
                nc.vector.tensor_add(
                    v3[:, h0:h0 + nhc, 0:64],
                    ps[:].rearrange("p (h c) -> p h c", c=64),
                    b_v_sb[:, o0:o0 + oc].rearrange("p (h c) -> p h c", c=64),
                )

            def emit_c_group(b, tt, e0, use_sc=False):
                # output projection chunk + bias + store. use_sc: borrow an
                # idle scores-pool tile as the accumulator (only legal after
                # all attention is emitted) for a deeper drain pipeline.
                ec = min(TC, dim - e0)
                if use_sc:
                    ps = ps_sc.tile([128, 2 * TC], F32, tag="sc",
                                    name="ps")[:, 0:ec]
                else:
                    ps = ps_mm.tile([128, ec], F32, tag="mm", name="ps")
                for dc in range(n_dc):
                    nc.tensor.matmul(
                        ps[:],
                        lhsT=outT[b][dc][:, tt * 128:(tt + 1) * 128],
                        rhs=wp_sb[:, dc * dim + e0:dc * dim + e0 + ec],
                        start=(dc == 0),
                        stop=(dc == n_dc - 1),
                    )
                yt = p_y.tile([128, ec], BF16, tag="y", name="yt")
                nc.vector.tensor_add(yt[:], ps[:], b_p_sb[:, e0:e0 + ec])
                nc.sync.dma_start(
                    out_d[b, tt * 128:(tt + 1) * 128, e0:e0 + ec], yt[:]
                )

            def emit_tail_dma(job):
                # deferred one iteration: SWDGE moves the denominator rows
                # from partition 64 down to partition-0 tiles (gpsimd ops
                # and custom-DVE ops misread partition-offset sources on
                # HW, so both the recip and the broadcast need base-0 APs)
                cps = job["cps"]
                d0s = []
                for sub in range(2):
                    d0 = p_den.tile([1, TC], F32, tag="d0", name="d0")
                    nc.gpsimd.dma_start(d0[:], cps[sub][64:65, :])
                    d0s.append(d0)
                job["d0s"] = d0s

            def emit_tail_rest(job):
                # deferred two iterations: single-pass approx-reciprocal of
                # the [1, 512] denominator rows (~51 ULP, plenty for a bf16
                # output), then GpSimd Q7-ring broadcast to 64 partitions.
                # partition_broadcast instead of a 0-stride-source DMA: the
                # DMA form reads the single source partition 64x and
                # serializes ~6us on its SBUF port; the Q7 ring distributes
                # it in ~1us
                rbs = []
                for sub in range(2):
                    rc = p_den.tile([1, TC], F32, tag="rc", name="rc")
                    nc.vector.reciprocal_approx_fast(rc[:], job["d0s"][sub][:])
                    rb = p_bc.tile([64, TC], F32, tag="bcast", name="rb")
                    nc.gpsimd.partition_broadcast(rb[:], rc[:])
                    rbs.append(rb)
                job["rbs"] = rbs

            def emit_tail_muls(job):
                # deferred ~3 iterations: by now the broadcast+recip are
                # done, so these never head-of-line block the DVE queue
                b, hp, i0 = job["b"], job["hp"], job["i0"]
                cps, rbs = job["cps"], job["rbs"]
                o_tile = outT[b][hp]
                tmp = p_sm.tile([64, TC], BF16, tag="ntmp", name="tmp")
                nc.vector.tensor_mul(tmp[:], cps[1][0:64, :], rbs[1][:])
                # SWDGE: HWDGE direct2d DMAs carry at most one sync wait and
                # this partition-shifting copy needs two. Emitted before mul0
                # so the copy overlaps it instead of trailing it.
                nc.gpsimd.dma_start(o_tile[64:128, i0:i0 + TC], tmp[:])
                nc.vector.tensor_mul(
                    o_tile[0:64, i0:i0 + TC], cps[0][0:64, :], rbs[0][:]
                )

            def emit_tail(b, hp, i0, po):
                # normalize: out[d, i] = po[d, i] / po[64, i].
                # Phase 1 (here): DVE evacuates the PV accumulators, freeing
                # the PSUM banks for the next segment. The broadcasts,
                # reciprocal, and multiplies are deferred (emit_tail_rest /
                # emit_tail_muls) so no queue ever waits at its head on
                # another queue's in-flight op.
                if outT[b][hp] is None:
                    outT[b][hp] = p_out.tile([128, t], BF16, tag="outT",
                                             name="ot")
                cps = []
                for sub in range(2):
                    cp = p_cp.tile([65, TC], F32, tag="cp", name="cp")
                    nc.vector.tensor_copy(cp[:], po[sub][:])
                    cps.append(cp)
                return {"b": b, "hp": hp, "i0": i0, "cps": cps}

            def emit_attention(b, fillers, i0_outer=False):
                """Attention for batch b; fillers = list of closures emitted
                at an even pace between iterations (PE filler work)."""
                if i0_outer:
                    segs = [(hp, i0) for i0 in range(0, t, TC)
                            for hp in range(n_hp)]
                else:
                    segs = [(hp, i0) for hp in range(n_hp)
                            for i0 in range(0, t, TC)]
                iters = [(hp, i0, jt) for hp, i0 in segs
                         for jt in range(n_tt)]
                n_it = len(iters)
                fillers = sorted(fillers, key=lambda p: p[0])
                fq = list(fillers)
                stash_q = []
                seg_po = None
                pending = []
                depth = 3
                for n in range(n_it + depth):
                    new_stash = None
                    if n < n_it:
                        hp, i0, jt = iters[n]
                        q_tile = qk_t[b][(hp, "q")]
                        k_tile = qk_t[b][(hp, "k")]
                        st = ps_sc.tile([128, 2 * TC], F32, tag="sc", name="st")
                        for sub in range(2):
                            nc.tensor.matmul(
                                st[:, sub * TC:(sub + 1) * TC],
                                lhsT=k_tile[sub * 64:(sub + 1) * 64,
                                            jt * 128:(jt + 1) * 128],
                                rhs=q_tile[sub * 64:(sub + 1) * 64,
                                           i0:i0 + TC],
                                start=True,
                                stop=True,
                            )
                        et = p_exp.tile([128, 2 * TC], BF16, tag="et",
                                        name="et")
                        nc.scalar.activation(
                            et[:], st[:], mybir.ActivationFunctionType.Exp,
                            scale=scale,
                        )
                        new_stash = (et, hp, i0, jt)
                    if new_stash is not None:
                        stash_q.append(new_stash)
                    # PV trails scores/exp by two iterations so it never
                    # waits on the exp that feeds it
                    if len(stash_q) > depth or (n >= n_it and stash_q):
                        et, hp, i0, jt = stash_q.pop(0)
                        if jt == 0:
                            seg_po = [
                                ps_o.tile([65, TC], F32, tag="po", name="po0"),
                                ps_o.tile([65, TC], F32, tag="po", name="po1"),
                            ]
                        for sub in range(2):
                            h = 2 * hp + sub
                            nc.tensor.matmul(
                                seg_po[sub][:],
                                lhsT=v_t[b][jt][:, h * 65:(h + 1) * 65],
                                rhs=et[:, sub * TC:(sub + 1) * TC],
                                start=(jt == 0),
                                stop=(jt == n_tt - 1),
                            )
                        while pending and pending[0][0] <= n:
                            pending.pop(0)[1]()
                        if jt == n_tt - 1:
                            job = emit_tail(b, hp, i0, seg_po)
                            pending.append(
                                (n + 1, lambda j=job: emit_tail_dma(j)))
                            pending.append(
                                (n + 2, lambda j=job: emit_tail_rest(j)))
                            pending.append(
                                (n + 4, lambda j=job: emit_tail_muls(j)))
                    # deadline-paced PE filler (after the tail block so
                    # the PSUM-freeing copies stay at the DVE queue head)
                    # 1 group/slot keeps filler coverage wide (a dry slot
                    # runs exp-paced, wasting ~220ns of PE); catch up at 2
                    # when the queue head is a slot overdue
                    emitted = 0
                    cap = 2 if (fq and fq[0][0] <= n - 1) else 1
                    while fq and fq[0][0] <= n and emitted < cap:
                        fq.pop(0)[1]()
                        emitted += 1
                while pending:
                    pending.pop(0)[1]()
                return fq  # un-emitted fillers (scheduled past the end)

            # ---- emission schedule ----
            # A: pair-0 q/k (i0=0) + v tt0 upfront — everything else is
            # deadline-paced filler inside the two attention phases.
            emit_qk_group(0, 0, "q", 0)
            emit_qk_group(0, 0, "k", 0)

            # Phase B0 fillers (batch-0 attention, hp-outer: pair k's first
            # segment starts at slot 16k; v0 tt_j consumed at slot j+2).
            # Keys lower-bound emission by estimated DMA arrival; the
            # 2-groups/slot cap paces actual consumption greedily so the
            # filler queue never runs dry mid-phase.
            # Early fillers in EXPLICIT order (sort is stable, so append
            # order breaks key ties): v0 tt_j must complete by slot j+2,
            # k(0,TC) by slot 4 (scores jt=4 reads k cols 512+), q(0,TC) by
            # slot 8 (the i0=512 segment).
            # B0 is i0-outer like B1: all pairs' i0=0 tails complete by
            # slot ~54, so the first half of batch-0's projection becomes
            # late-B0 filler instead of idling the PE in slots 80-95.
            fill_b0 = []
            early = []
            for tt in range(n_tt):
                for o0 in range(0, dim, TC):
                    early.append(lambda tt=tt, o0=o0: emit_v_group(0, tt, o0))
            early.insert(4, lambda: emit_qk_group(0, 0, "k", TC))
            for idx, f in enumerate(early):
                fill_b0.append((idx // 2, f))
            # qk0 pair k (i0-outer): q(0)/k(0) by slot 8k, k(TC) by 8k+4,
            # q(TC) by 48+8k. wra/wrb land before B0 starts (packed DMA).
            for k in range(1, n_hp):
                base = max(2, 8 * k - 8)
                for j, (which, i0) in enumerate(
                        (("q", 0), ("k", 0), ("k", TC))):
                    fill_b0.append((base + (0, 1, 3)[j],
                                    lambda k=k, which=which, i0=i0:
                                    emit_qk_group(0, k, which, i0)))
            for k in range(n_hp):
                fill_b0.append((28 + 5 * k,
                                lambda k=k: emit_qk_group(0, k, "q", TC)))
            # batch-1 qkv consumed at the START of B1 -> v1 + qk1 pairs 0-2
            # front-load into B0's spare slots; pairs 3-5 and q(i0=512)
            # keep slack into B1.
            others = []
            for tt in range(n_tt):
                for o0 in range(0, dim, TC):
                    others.append(lambda tt=tt, o0=o0: emit_v_group(1, tt, o0))
            for k in range(3):
                others.append(lambda k=k: emit_qk_group(1, k, "q", 0))
                others.append(lambda k=k: emit_qk_group(1, k, "k", 0))
                others.append(lambda k=k: emit_qk_group(1, k, "k", TC))
            n_o = len(others)
            for idx, f in enumerate(others):
                if idx < n_o - 5:
                    key = 18 + round((idx + 0.5) * 52 / (n_o - 5))
                else:
                    key = 84 + 2 * (idx - (n_o - 5))
                fill_b0.append((key, f))
            # batch-0 projection first half as late-B0 filler
            for gi, tt in enumerate(range(4)):
                for k0, e0 in enumerate(range(0, dim, TC)):
                    fill_b0.append((58 + 8 * gi + 4 * k0,
                                    lambda tt=tt, e0=e0: emit_c_group(0, tt, e0)))

            leftovers = emit_attention(0, fill_b0, i0_outer=True)

            # Phase B1 fillers: batch-1 qk pairs 3-5 by slot 8k; q(i0=512)
            # by slot 48+8k; batch-0 projection spread across the phase
            # (4 groups held back to cover the final tail-chain latency in
            # the drain); batch-1 projection first half (output rows 0-511,
            # complete after slot ~52) as late filler.
            fill_b1 = [(0, f[1]) for f in leftovers]
            for k in range(3, n_hp):
                fill_b1.append((8 * k - 20,
                                lambda k=k: emit_qk_group(1, k, "q", 0)))
                fill_b1.append((8 * k - 18,
                                lambda k=k: emit_qk_group(1, k, "k", 0)))
                fill_b1.append((8 * k - 16,
                                lambda k=k: emit_qk_group(1, k, "k", TC)))
            for k in range(n_hp):
                fill_b1.append((30 + 7 * k,
                                lambda k=k: emit_qk_group(1, k, "q", TC)))
            pj0 = [(tt, e0) for tt in range(4, n_tt)
                   for e0 in range(0, dim, TC)]
            pj0_keys = [26, 44]
            for key, (tt, e0) in zip(pj0_keys, pj0[:2]):
                fill_b1.append((key,
                                lambda tt=tt, e0=e0: emit_c_group(0, tt, e0)))
            for gi, tt in enumerate(range(4)):
                for k0, e0 in enumerate(range(0, dim, TC)):
                    fill_b1.append((66 + 6 * gi + 3 * k0,
                                    lambda tt=tt, e0=e0: emit_c_group(1, tt, e0)))

            left1 = emit_attention(1, fill_b1, i0_outer=True)
            for _, f in left1:
                f()
            # drain: batch-1 projection second half, interleaved with the
            # reserved batch-0 groups (no dependencies) that keep the PE
            # busy while the final segments' normalization chains land; the
            # batch-1 groups borrow idle scores-pool banks so they never
            # wait on the last filler drains stuck behind the flushed tails
            c_rest = [(1, tt, e0) for tt in range(4, n_tt)
                      for e0 in range(0, dim, TC)]
            c_cover = [(0, tt, e0) for tt, e0 in pj0[2:]]
            order = c_cover + c_rest
            for gi, (b, tt, e0) in enumerate(order):
                emit_c_group(b, tt, e0, use_sc=(gi % 2 == 1))

    nc.compile()
    return nc


def make_in_maps(x, w_qkv, b_qkv, w_proj, b_proj):
    import ml_dtypes

    bf16 = np.dtype(ml_dtypes.bfloat16)
    x = np.asarray(x, dtype=np.float32)
    w_qkvT = np.ascontiguousarray(np.asarray(w_qkv, np.float32).T)  # [d, 3d]

    def pack(w):
        # [768, C] -> [128, 6*C]: contraction chunks contiguous per
        # partition so the whole tensor loads as one big-packet DMA
        C = w.shape[1]
        return np.ascontiguousarray(
            w.reshape(6, 128, C).transpose(1, 0, 2).reshape(128, 6 * C))

    # hqk = [q_p0 | k_p0]; hv = v; ra = pairs 1-2; rb = pairs 3-5
    w_hqk = pack(np.concatenate(
        [w_qkvT[:, 0:128], w_qkvT[:, DIM:DIM + 128]], axis=1))
    w_hv = pack(w_qkvT[:, 2 * DIM:])
    rest = []
    for hp in range(1, NH // 2):
        rest.append(w_qkvT[:, hp * 128:(hp + 1) * 128])
        rest.append(w_qkvT[:, DIM + hp * 128:DIM + (hp + 1) * 128])
    w_ra = pack(np.concatenate(rest[:4], axis=1))
    w_rb = pack(np.concatenate(rest[4:], axis=1))
    w_projT = pack(np.ascontiguousarray(np.asarray(w_proj, np.float32).T)).astype(bf16)
    b_qkv = np.asarray(b_qkv, np.float32)
    b_qkT = np.ascontiguousarray(b_qkv[:2 * DIM].reshape(2 * DIM // 128, 128).T)
    b_v = np.ascontiguousarray(np.broadcast_to(b_qkv[2 * DIM:], (128, DIM))).astype(bf16)
    b_p = np.ascontiguousarray(
        np.broadcast_to(np.asarray(b_proj, np.float32), (128, DIM))).astype(bf16)
    w_hqk = np.ascontiguousarray(w_hqk).astype(bf16)
    w_hv = np.ascontiguousarray(w_hv).astype(bf16)
    w_ra = np.ascontiguousarray(w_ra).astype(bf16)
    w_rb = np.ascontiguousarray(w_rb).astype(bf16)
    in_maps = []
    for c in range(N_CORES):
        xs = x[c * B_LOC:(c + 1) * B_LOC]
        xT = np.ascontiguousarray(xs.transpose(0, 2, 1))  # [b, 768, 1024]
        xP = np.stack([
            np.stack([pack(xT[b, :, h * 512:(h + 1) * 512]) for h in range(2)])
            for b in range(B_LOC)
        ]).astype(bf16)
        in_maps.append({
            "xP": xP,
            "w_hqk": w_hqk,
            "w_hv": w_hv,
            "w_ra": w_ra,
            "w_rb": w_rb,
            "w_projT": w_projT,
            "b_qkT": b_qkT,
            "b_v": b_v,
            "b_proj": b_p,
        })
    return in_maps


_NC_CACHE = {}


def _get_nc():
    if "nc" not in _NC_CACHE:
        _NC_CACHE["nc"] = build_nc()
    return _NC_CACHE["nc"]


def run(x, w_qkv, b_qkv, w_proj, b_proj, **rb_kwargs):
    nc = _get_nc()
    in_maps = make_in_maps(x, w_qkv, b_qkv, w_proj, b_proj)
    res = run_bass_kernel_spmd(nc, in_maps, core_ids=list(range(N_CORES)), **rb_kwargs)
    out = np.concatenate([r["out"] for r in res.results], axis=0)
    return out.astype(np.float32), res


def kernel(x, w_qkv, b_qkv, w_proj, b_proj):
    out, _ = run(x, w_qkv, b_qkv, w_proj, b_proj)
    return out
